# revision 51
# baseline (speedup 1.0000x reference)
"""Trainium2 Bass kernel for causal multi-head attention.

Problem: B=4, T=2048, D=1024, H=16, HD=64, fp32, causal, scale=1/sqrt(D).

Sharding: 4-way batch x 2-way head-group over 8 cores. Core c=(b,g) computes
heads g*8..g*8+7 for batch b and returns the partial output projection
(contracted over its 512 context columns) in bf16; the host sums the two
partials per batch element and adds bo.

Per-core dataflow:
  - QKV projections run as 3-term fp8 DoubleRow matmuls (x_hi@W_hi + x_hi@W_lo
    + x_lo@W_hi), with the hi/lo e4m3 split done on the host. DoubleRow
    contracts 256 rows at 0.5 cycles/row, so each projection costs 25% less
    PE time than the f32r version while keeping ~bf16 accuracy. x is
    pre-scaled by 8 and W by 32 so both the hi values and the residuals sit
    in e4m3's normal range (unscaled residuals underflow into subnormals and
    the correction terms become noise); the 1/256 product scale is undone in
    the Q/K copyback, and for V it cancels against the softmax denominator
    (ones column and bv both carry 256).
  - Q^T/K^T land in SBUF as bf16 [feat_part, token]; biases are added during
    the PSUM->SBUF copyback (DVE). The 1/sqrt(D) softmax scale is folded into
    the exp activation, not the projection.
  - Scores are computed transposed, S^T[tk_part, tq_free] (bf16 matmul), two
    tk blocks per 2-bank PSUM tile so exp covers 1024 elements per scalar-
    engine instruction. Causal masking adds -4096 via one bf16 matmul over
    exactly the 128 columns of the diagonal band (the mask pattern is j<p for
    every diagonal block, independent of tile indices).
  - PV runs transposed: ctx[tq_part, d] = pt^T @ V with a 65-wide moving
    operand (64 v-channels + a ones column that accumulates the softmax
    denominator). This costs 65 PE rows per (128x128 tq,tk) block instead of
    512, and the denominator arrives per-partition, so normalization is a
    reciprocal + per-head tensor_scalar with no DRAM broadcast. GPSIMD cannot
    access PSUM, so all copybacks stay on the DVE.
  - ctx tiles are transposed back to [feat_part, token] via DMA XBAR
    transposes (idle DMA capacity) feeding the bf16 output projection.
  - Emission is software-pipelined at head-pair (= ft tile) granularity: the
    PV/normalize/transpose units of the previous ft tile, the next tq tile's
    projection quanta (~640ns chunks sharing PSUM pair tiles), and the
    previous tq tile's output-projection quanta are woven between score/exp
    units on a proportional pace so the scalar engine stays fed. x loads
    issue from the Activation queue so a blocked transpose on the SP queue
    can never delay them (head-of-line convoy).
"""

import numpy as np
from contextlib import ExitStack

import ml_dtypes
import concourse.bass as bass
import concourse.tile as tile
from concourse import bacc
from concourse import mybir
from concourse.bass_utils import run_bass_kernel_spmd

F32 = mybir.dt.float32
BF16 = mybir.dt.bfloat16
FP8 = mybir.dt.float8e4
AF = mybir.ActivationFunctionType
OP = mybir.AluOpType
DR = mybir.MatmulPerfMode.DoubleRow

E4 = ml_dtypes.float8_e4m3
BF = ml_dtypes.bfloat16

MASKVAL = -4096.0  # exp((s-4096)/32) == 0 in fp32 for any realistic s


def build_mha_core(T, D, F, DOUT, HD=64, TQ=512, num_devices=1):
    """Build the per-core Bass program.

    T: tokens, D: model dim, F: feature columns owned by this core,
    DOUT: output projection width, HD: head dim, TQ: tq tile width.
    """
    NH = F // HD        # local heads (8)
    DT2 = D // 256      # DoubleRow contraction tiles (4)
    FT = F // 128       # feature 128-tiles (4)
    NTOK = T // 128     # token 128-tiles (16)
    NTQ = T // TQ       # tq tiles (4)
    NR = TQ // 128      # 128-blocks per tq tile (4)
    NCH = min(512, DOUT)
    NO = DOUT // NCH
    NG = NH // 4        # head groups of 4 (2)

    nc = bacc.Bacc(None, target_bir_lowering=False, debug=False, num_devices=num_devices)

    xq_hi = nc.dram_tensor("xq_hi", [D, T], FP8, kind="ExternalInput")
    xq_lo = nc.dram_tensor("xq_lo", [D, T], FP8, kind="ExternalInput")
    xk_hi = nc.dram_tensor("xk_hi", [D, T], FP8, kind="ExternalInput")
    xk_lo = nc.dram_tensor("xk_lo", [D, T], FP8, kind="ExternalInput")
    xv_hi = nc.dram_tensor("xv_hi", [D, T], FP8, kind="ExternalInput")
    xv_lo = nc.dram_tensor("xv_lo", [D, T], FP8, kind="ExternalInput")
    Wq_hi = nc.dram_tensor("Wq_hi", [128, DT2, 2, F], FP8, kind="ExternalInput")
    Wq_lo = nc.dram_tensor("Wq_lo", [128, DT2, 2, F], FP8, kind="ExternalInput")
    Wk_hi = nc.dram_tensor("Wk_hi", [128, DT2, 2, F], FP8, kind="ExternalInput")
    Wk_lo = nc.dram_tensor("Wk_lo", [128, DT2, 2, F], FP8, kind="ExternalInput")
    Wv_hi = nc.dram_tensor("Wv_hi", [128, DT2, 2, F], FP8, kind="ExternalInput")
    Wv_lo = nc.dram_tensor("Wv_lo", [128, DT2, 2, F], FP8, kind="ExternalInput")
    Wo = nc.dram_tensor("Wo", [128, FT, DOUT], BF16, kind="ExternalInput")
    bq = nc.dram_tensor("bq", [128, FT], F32, kind="ExternalInput")
    bk = nc.dram_tensor("bk", [128, FT], F32, kind="ExternalInput")
    bv = nc.dram_tensor("bv", [1, F], F32, kind="ExternalInput")
    ones = nc.dram_tensor("ones", [1, 1], BF16, kind="ExternalInput")
    mneg = nc.dram_tensor("mneg", [128, 128], BF16, kind="ExternalInput")
    ident = nc.dram_tensor("ident", [128, 128], BF16, kind="ExternalInput")
    out = nc.dram_tensor("out", [T, DOUT], BF16, kind="ExternalOutput")

    with tile.TileContext(nc) as tc:
        with ExitStack() as ctx:
            persist = ctx.enter_context(tc.tile_pool(name="persist", bufs=1))
            QT_sb = persist.tile([128, FT, T], BF16)
            KT_sb = persist.tile([128, FT, T], BF16)
            VA_sb = persist.tile([128, NTOK, NH, HD + 1], BF16)
            CTXT_sb = persist.tile([128, FT, T], BF16)
            Wo_sb = persist.tile([128, FT, DOUT], BF16)
            bq_sb = persist.tile([128, FT], F32)
            bk_sb = persist.tile([128, FT], F32)
            bv_sb = persist.tile([128, F], F32)
            mneg_sb = persist.tile([128, 128], BF16)
            ident_sb = persist.tile([128, 128], BF16)
            Wsb = {}
            for nm in ("q", "k", "v"):
                Wsb[nm] = (persist.tile([128, DT2, 2, F], FP8, name=f"W{nm}h"),
                           persist.tile([128, DT2, 2, F], FP8, name=f"W{nm}l"))

            # PSUM: pwork (3 x 2-bank tiles, shared by projections / score
            # pairs / out-proj) + psC (2 x 1-bank) = 8 banks exactly.
            pwork = ctx.enter_context(tc.tile_pool(name="pwork", bufs=3, space="PSUM"))
            psC = ctx.enter_context(tc.tile_pool(name="psC", bufs=2, space="PSUM"))

            xpool = ctx.enter_context(tc.tile_pool(name="xin", bufs=8))
            ptpool = ctx.enter_context(tc.tile_pool(name="pt", bufs=6))
            ctxpool = ctx.enter_context(tc.tile_pool(name="ctx", bufs=8))
            denpool = ctx.enter_context(tc.tile_pool(name="den", bufs=8))
            outpool = ctx.enter_context(tc.tile_pool(name="osb", bufs=3))

            WD = {"q": (Wq_hi, Wq_lo), "k": (Wk_hi, Wk_lo), "v": (Wv_hi, Wv_lo)}
            XD = {"q": (xq_hi, xq_lo), "k": (xk_hi, xk_lo), "v": (xv_hi, xv_lo)}

            def load_w(nm):
                for sb, dram in zip(Wsb[nm], WD[nm]):
                    nc.sync.dma_start(sb[:], dram[:])

            def load_x(nm, tj):
                ts = []
                for dram in XD[nm]:
                    t_ = xpool.tile([128, DT2, 2, TQ], FP8, tag="xin")
                    src = dram[:].rearrange("(dt s p) t -> p dt s t", p=128, s=2)
                    nc.scalar.dma_start(t_[:], src[:, :, :, tj * TQ:(tj + 1) * TQ])
                    ts.append(t_)
                return ts

            def dr_chain(ps, lhs_of, rhs_of):
                """3-term DoubleRow chain: (hi,hi), (hi,lo), (lo,hi)."""
                terms = [(0, 0), (0, 1), (1, 0)]
                n = len(terms) * DT2
                j = 0
                for xi, wi in terms:
                    for dt in range(DT2):
                        nc.tensor.matmul(
                            ps, lhsT=lhs_of(xi, wi, dt), rhs=rhs_of(xi, wi, dt),
                            start=(j == 0), stop=(j == n - 1), perf_mode=DR)
                        j += 1

            def qk_chain(nm, tj, xt, ft, ps, slot):
                """Two ~640ns quanta: first 6 DR matmuls, then 6 + copyback."""
                dst = QT_sb if nm == "q" else KT_sb
                bsb = bq_sb if nm == "q" else bk_sb
                wh, wl = Wsb[nm]
                terms = [(0, 0), (0, 1), (1, 0)]
                mms = [(xi, wi, dt) for xi, wi in terms for dt in range(DT2)]

                def emit(half):
                    for j in range(6 * half, 6 * half + 6):
                        xi, wi, dt = mms[j]
                        nc.tensor.matmul(
                            ps[:, slot, :],
                            lhsT=(wh if wi == 0 else wl)[:, dt, :, ft * 128:(ft + 1) * 128],
                            rhs=xt[xi][:, dt, :, :],
                            start=(j == 0), stop=(j == 11), perf_mode=DR)
                    if half == 1:
                        nc.vector.tensor_scalar(
                            dst[:, ft, tj * TQ:(tj + 1) * TQ], ps[:, slot, :],
                            1.0 / 256, bsb[:, ft:ft + 1], OP.mult, OP.add)
                return [lambda: emit(0), lambda: emit(1)]

            def v_chain(tj, xt, c, ps, slot):
                tt = tj * NR + c
                wh, wl = Wsb["v"]
                terms = [(0, 0), (0, 1), (1, 0)]
                mms = [(xi, wi, dt) for xi, wi in terms for dt in range(DT2)]
                psv = ps[:, slot, :F]

                def emit(half):
                    for j in range(6 * half, 6 * half + 6):
                        xi, wi, dt = mms[j]
                        nc.tensor.matmul(
                            psv,
                            lhsT=xt[xi][:, dt, :, c * 128:(c + 1) * 128],
                            rhs=(wh if wi == 0 else wl)[:, dt, :, :],
                            start=(j == 0), stop=(j == 11), perf_mode=DR)
                    if half == 1:
                        nc.vector.tensor_tensor(
                            VA_sb[:, tt, :, 0:HD],
                            psv.rearrange("p (h d) -> p h d", h=NH),
                            bv_sb[:].rearrange("p (h d) -> p h d", h=NH),
                            OP.add)
                return [lambda: emit(0), lambda: emit(1)]

            def out_chain(tt, split_copy=False):
                """Four ~430ns matmul quanta + copy/store."""
                ot = outpool.tile([128, NO, NCH], BF16, tag="ot", name="ot")
                ps = pwork.tile([128, 2, NCH], F32, tag="pwork", name="psout")

                def emit(n, fth):
                    for ft in (2 * fth, 2 * fth + 1):
                        nc.tensor.matmul(
                            ps[:, n, :],
                            lhsT=CTXT_sb[:, ft, tt * 128:(tt + 1) * 128],
                            rhs=Wo_sb[:, ft, n * NCH:(n + 1) * NCH],
                            start=(ft == 0), stop=(ft == FT - 1))

                def fin():
                    nc.vector.tensor_copy(
                        ot[:].rearrange("p a b -> p (a b)"),
                        ps[:].rearrange("p a b -> p (a b)"))
                    nc.sync.dma_start(
                        out[tt * 128:(tt + 1) * 128, :],
                        ot[:].rearrange("p a b -> p (a b)"))
                return [lambda: emit(0, 0), lambda: emit(0, 1),
                        lambda: emit(1, 0), lambda: (emit(1, 1), fin())]

            def s_units(ft, tj, pts):
                """Score+exp unit closures for both heads of ft tile. Each
                unit emits one 2-block PSUM pair and its exp(s), appending
                probability-tile APs to pts[z]."""
                units = []
                for z in range(2):
                    h = 2 * ft + z
                    po = (h % 2) * HD
                    QhT = QT_sb[po:po + HD, ft, :]
                    KhT = KT_sb[po:po + HD, ft, :]

                    def full(i2, QhT=QhT, KhT=KhT, z=z):
                        psp = pwork.tile([128, 2, TQ], F32, tag="pwork", name="psS2")
                        for s in range(2):
                            nc.tensor.matmul(
                                psp[:, s, :],
                                lhsT=KhT[:, (i2 + s) * 128:(i2 + s + 1) * 128],
                                rhs=QhT[:, tj * TQ:(tj + 1) * TQ],
                                start=True, stop=True)
                        pt = ptpool.tile([128, 2, TQ], BF16, tag="pt2", name="pt2",
                                         bufs=33)
                        nc.scalar.activation(pt[:], psp[:], AF.Exp, scale=1.0 / 32)
                        pts[z] += [pt[:, 0, :], pt[:, 1, :]]

                    def diag(r2, QhT=QhT, KhT=KhT, z=z):
                        psp = pwork.tile([128, 2, TQ], F32, tag="pwork", name="psSd")
                        pt = ptpool.tile([128, 2, TQ], BF16, tag="pt2", name="ptd",
                                         bufs=33)
                        for s in range(2):
                            r = r2 + s
                            c0 = 128 * r
                            nc.tensor.matmul(
                                psp[:, s, c0:],
                                lhsT=KhT[:, (NR * tj + r) * 128:(NR * tj + r + 1) * 128],
                                rhs=QhT[:, tj * TQ + c0:(tj + 1) * TQ],
                                start=True, stop=False)
                            nc.tensor.matmul(
                                psp[:, s, c0:c0 + 128],
                                lhsT=ident_sb[:],
                                rhs=mneg_sb[:],
                                start=False, stop=True, skip_group_check=True)
                            nc.scalar.activation(pt[:, s, c0:], psp[:, s, c0:],
                                                 AF.Exp, scale=1.0 / 32)
                            pts[z].append(pt[:, s, :])

                    units += [(lambda i2=i2, f=full: f(i2))
                              for i2 in range(0, NR * tj, 2)]
                    units += [(lambda r2=r2, f=diag: f(r2))
                              for r2 in range(0, NR, 2)]
                return units

            def pv_units(ft, tj, pts):
                """PV + normalize + transpose closures, one per tq sub-block."""
                def rs_unit(rs):
                    jb = NR * tj + rs
                    # one full, padded PSUM bank per accumulation group with
                    # an unconditional start: matmul start zeroing is
                    # bank-granular on HW, so groups must never share a bank
                    psC_t = [psC.tile([128, HD + 1], F32, tag="psC",
                                      name=f"psC{z}", padded_shape=[128, 512])
                             for z in range(2)]
                    for z in range(2):
                        h = 2 * ft + z
                        for i in range(jb + 1):
                            nc.tensor.matmul(
                                psC_t[z][:],
                                lhsT=pts[z][i][:, rs * 128:(rs + 1) * 128],
                                rhs=VA_sb[:, i, h, :],
                                start=(i == 0), stop=(i == jb))
                    den = denpool.tile([128, 2, 1], F32, tag="den")
                    ctx_t = ctxpool.tile([128, 2, HD], BF16, tag="ctx")
                    # GPSIMD cannot touch PSUM, so everything PSUM-side
                    # lives on the DVE
                    for z in range(2):
                        nc.vector.reciprocal(den[:, z, :],
                                             psC_t[z][:, HD:HD + 1])
                        nc.vector.tensor_scalar_mul(
                            ctx_t[:, z, :], psC_t[z][:, 0:HD], den[:, z, :])
                    nc.sync.dma_start_transpose(
                        CTXT_sb[:, ft, jb * 128:(jb + 1) * 128], ctx_t[:])
                return [(lambda rs=rs: rs_unit(rs)) for rs in range(NR)]

            # ---- prologue: weights + tj=0 projections. DMA order is the
            # startup critical path (HWDGE serializes): W/x interleaved per
            # tensor in q, k, v order so the first score matmul can start as
            # early as possible; small constant DMAs woven where needed. ----
            xt0 = {}
            load_w("q")
            xt0["q"] = load_x("q", 0)
            nc.sync.dma_start(mneg_sb[:], mneg[:])
            nc.sync.dma_start(ident_sb[:], ident[:])
            load_w("k")
            xt0["k"] = load_x("k", 0)
            nc.sync.dma_start(bq_sb[:], bq[:])
            nc.sync.dma_start(bk_sb[:], bk[:])
            load_w("v")
            xt0["v"] = load_x("v", 0)
            nc.sync.dma_start(bv_sb[:], bv[:].to_broadcast([128, F]))
            nc.sync.dma_start(
                VA_sb[:].rearrange("p a b c -> p (a b) c")[:, :, HD:HD + 1],
                ones[0:1, 0:1].to_broadcast([128, NTOK * NH, 1]))
            for ftp in range(0, FT, 2):
                for nm in ("q", "k", "v"):
                    ps = pwork.tile([128, 2, TQ], F32, tag="pwork", name="psp0")
                    for slot in range(2):
                        if nm == "v":
                            qs = v_chain(0, xt0[nm], ftp + slot, ps, slot)
                        else:
                            qs = qk_chain(nm, 0, xt0[nm], ftp + slot, ps, slot)
                        for q_ in qs:
                            q_()
            nc.sync.dma_start(Wo_sb[:], Wo[:])

            # ---- attention, software-pipelined at head-pair level: the PV/
            # finalize units of the previous ft tile, next-tile projection
            # quanta, and previous-tile output-projection quanta are woven
            # between score/exp units so no engine starves. ----
            pend_pv = []
            for tj in range(NTQ):
                fillers = []
                if tj + 1 < NTQ:
                    xt = {}
                    tj1 = tj + 1

                    def mk_load(nm, hl, tj1=tj1):
                        def f(nm=nm, hl=hl):
                            dram = XD[nm][hl]
                            t_ = xpool.tile([128, DT2, 2, TQ], FP8, tag="xin",
                                            name="xin")
                            src = dram[:].rearrange("(dt s p) t -> p dt s t",
                                                    p=128, s=2)
                            nc.scalar.dma_start(
                                t_[:], src[:, :, :, tj1 * TQ:(tj1 + 1) * TQ])
                            xt.setdefault(nm, [None, None])[hl] = t_
                        return f

                    def mk_pair(nm, j2, tj1=tj1):
                        """4 quanta covering chains j2, j2+1 of projection nm,
                        sharing one PSUM pair tile."""
                        state = {}

                        def q_of(slot, half):
                            def f():
                                if "ps" not in state:
                                    state["ps"] = pwork.tile(
                                        [128, 2, TQ], F32, tag="pwork",
                                        name="psproj")
                                key = slot
                                if key not in state:
                                    if nm == "v":
                                        state[key] = v_chain(
                                            tj1, xt["v"], j2 + slot,
                                            state["ps"], slot)
                                    else:
                                        state[key] = qk_chain(
                                            nm, tj1, xt[nm], j2 + slot,
                                            state["ps"], slot)
                                state[key][half]()
                            return f
                        return [q_of(0, 0), q_of(0, 1), q_of(1, 0), q_of(1, 1)]

                    outs = ([out_chain(tt) for tt in range((tj - 1) * NR, tj * NR)]
                            if tj > 0 else [[]] * 4)
                    fillers = (
                        [mk_load("v", 0), mk_load("v", 1), mk_load("q", 0)]
                        + outs[0]
                        + mk_pair("v", 0)
                        + [mk_load("q", 1)]
                        + outs[1]
                        + mk_pair("v", 2)
                        + [mk_load("k", 0)]
                        + mk_pair("q", 0)
                        + [mk_load("k", 1)]
                        + outs[2]
                        + mk_pair("q", 2)
                        + mk_pair("k", 0)
                        + outs[3]
                        + mk_pair("k", 2))
                elif tj > 0:
                    for tt in range((tj - 1) * NR, tj * NR):
                        fillers += out_chain(tt)
                nsu = 4 * (4 * tj + 4) // 2
                nfill = len(fillers)
                popped = [0]
                emitted = [0]

                def tick():
                    emitted[0] += 1
                    want = emitted[0] * nfill // nsu
                    while fillers and popped[0] < want:
                        popped[0] += 1
                        fillers.pop(0)()

                for ft in range(FT):
                    pts = {0: [], 1: []}
                    sus = s_units(ft, tj, pts)
                    pvstep = max(1, len(sus) // (len(pend_pv) + 1))
                    for idx, su in enumerate(sus):
                        su()
                        if pend_pv and (idx + 1) % pvstep == 0:
                            pend_pv.pop(0)()
                        tick()
                    while pend_pv:
                        pend_pv.pop(0)()
                    pend_pv = pv_units(ft, tj, pts)
                while fillers:
                    fillers.pop(0)()

            while pend_pv:
                pend_pv.pop(0)()

            # ---- output projection for the last tq tile ----
            for tt in range((NTQ - 1) * NR, NTOK):
                for q in out_chain(tt, split_copy=True):
                    q()

    nc.compile()
    return nc


SX = 8.0    # x pre-scale: keeps e4m3 hi AND residual in the normal range
SW = 32.0   # W pre-scale; the product scale 1/(SX*SW) is undone downstream


def _hilo(x, s):
    xs = np.asarray(x, np.float32) * np.float32(s)
    hi = xs.astype(E4)
    lo = (xs - hi.astype(np.float32)).astype(E4)
    return hi, lo


def _w_dr(W):
    """[D, F] -> hi/lo e4m3 in DoubleRow layout [128, DT2, 2, F]."""
    D, F = W.shape
    hi, lo = _hilo(W, SW)
    def lay(a):
        return np.ascontiguousarray(
            a.reshape(D // 256, 2, 128, F).transpose(2, 0, 1, 3))
    return lay(hi), lay(lo)


def make_core_inputs(q_b, k_b, v_b, Wq, bq, Wk, bk, Wv, bv, Wo, fsl):
    F = fsl.stop - fsl.start
    FT = F // 128
    p = np.arange(128)
    mneg = np.where(p[None, :] < p[:, None], np.float32(MASKVAL), np.float32(0.0))
    d = {}
    for nm, x in (("xq", q_b.T), ("xk", k_b.T), ("xv", v_b.T)):
        hi, lo = _hilo(np.ascontiguousarray(x, dtype=np.float32), SX)
        d[nm + "_hi"], d[nm + "_lo"] = hi, lo
    for nm, W in (("Wq", Wq[:, fsl]), ("Wk", Wk[:, fsl]), ("Wv", Wv[:, fsl])):
        d[nm + "_hi"], d[nm + "_lo"] = _w_dr(W)
    Wo_l = np.asarray(Wo[fsl, :], np.float32)
    d["Wo"] = np.ascontiguousarray(
        Wo_l.reshape(FT, 128, -1).transpose(1, 0, 2)).astype(BF)
    d["bq"] = np.ascontiguousarray(np.asarray(bq[fsl], np.float32).reshape(FT, 128).T)
    d["bk"] = np.ascontiguousarray(np.asarray(bk[fsl], np.float32).reshape(FT, 128).T)
    d["bv"] = np.ascontiguousarray(
        256.0 * np.asarray(bv[fsl], np.float32).reshape(1, F))
    d["ones"] = np.full((1, 1), 256.0, np.float32).astype(BF)
    d["mneg"] = mneg.astype(BF)
    d["ident"] = np.eye(128, dtype=np.float32).astype(BF)
    return d


_CACHE = {}


def kernel(q, k, v, Wq, bq, Wk, bk, Wv, bv, Wo, bo, _trace=False):
    B, T, D = q.shape
    H, HD = 16, 64
    n_cores = 8
    gpb = n_cores // B            # head-groups per batch element (2)
    F = D // gpb                  # feature columns per core (512)

    key = (T, D, F)
    if key not in _CACHE:
        _CACHE[key] = build_mha_core(T=T, D=D, F=F, DOUT=D, HD=HD, TQ=512,
                                     num_devices=n_cores)
    nc = _CACHE[key]

    q = np.asarray(q, np.float32)
    k = np.asarray(k, np.float32)
    v = np.asarray(v, np.float32)
    in_maps = []
    for c in range(n_cores):
        b, g = c // gpb, c % gpb
        fsl = slice(g * F, (g + 1) * F)
        in_maps.append(make_core_inputs(
            q[b], k[b], v[b], Wq, bq, Wk, bk, Wv, bv, Wo, fsl))

    res = run_bass_kernel_spmd(nc, in_maps, list(range(n_cores)), trace=_trace)
    out = np.zeros((B, T, D), np.float32)
    for c in range(n_cores):
        out[c // gpb] += res.results[c]["out"].astype(np.float32)
    out += np.asarray(bo, np.float32)
    if _trace:
        kernel.last_exec_time_ns = res.exec_time_ns
    return out
